# revision 8
# baseline (speedup 1.0000x reference)
"""Trainium2 Bass kernel for nn_AttnDecoderRNN (B=128, H=1024, L=64, V=32000).

Strategy across 8 NeuronCores:
  - Attention (the 34 GFLOP ua_e einsum + scores + softmax + context) is
    data-parallel over batch: each core owns 16 batch rows.
  - LSTM cell is tensor-parallel over the hidden dim: ctx is all-gathered,
    then each core computes a 128-wide H-slice of the gates/c1/h1 with the
    matching column slice of W_ih/W_hh.
  - Output projection is column-parallel over the vocab: h1 is all-gathered
    (transposed), each core computes logits for 4000 vocab columns, and a
    tiny stats all-gather turns local max/sumexp into the global log_softmax.

Matmul operands travel as bfloat16 (fp32 accumulation in PSUM); everything
else (softmax, LSTM elementwise, log-softmax) stays fp32.
"""
import os
import sys

if "/opt/trn_rl_repo" not in sys.path:
    sys.path.insert(0, "/opt/trn_rl_repo")

import ml_dtypes
import numpy as np
import orjson

# ---------------------------------------------------------------------------
# This container's walrus build only supports ONE semaphore wait per
# instruction ("Too many sync wait commands").  Split any instruction carrying
# k>1 waits into (k-1) standalone EventSemaphore waits on the same engine
# immediately before it (per-engine program order is preserved).
# ---------------------------------------------------------------------------


def _split_multiwaits(bir_bytes: bytes) -> bytes:
    j = orjson.loads(bir_bytes)
    counter = 0
    changed = False
    for func in j.get("functions", []):
        for bb in func.get("blocks", []):
            insts = bb.get("instructions", [])
            new_insts = []
            for ins in insts:
                si = ins.get("sync_info")
                if si:
                    waits = si.get("on_wait") or []
                    if len(waits) > 1:
                        changed = True
                        for w in waits[:-1]:
                            counter += 1
                            new_insts.append(
                                {
                                    "debug": ins.get("debug"),
                                    "engine": ins["engine"],
                                    "ins": [],
                                    "name": f"WSPLIT-{counter}",
                                    "opcode": "EventSemaphore",
                                    "outs": [],
                                    "sync_info": {"on_update": [], "on_wait": [w]},
                                }
                            )
                        si["on_wait"] = [waits[-1]]
                new_insts.append(ins)
            if len(new_insts) != len(insts):
                bb["instructions"] = new_insts
    return orjson.dumps(j) if changed else bir_bytes


def _install_birfix():
    import concourse.bass as bass

    if getattr(bass.Bass.to_json_bytes, "_wsplit_patched", False):
        return
    orig = bass.Bass.to_json_bytes

    def to_json_bytes(self):
        return _split_multiwaits(orig(self))

    to_json_bytes._wsplit_patched = True
    bass.Bass.to_json_bytes = to_json_bytes


_install_birfix()

import concourse.bass as bass
import concourse.tile as tile
from concourse import mybir
from concourse.bass_utils import run_bass_kernel_spmd
from concourse.masks import make_identity

# ---------------------------------------------------------------------------
# Problem constants (hardcoded per the harness contract).
# ---------------------------------------------------------------------------
NCORES = 8
B, H, L, V = 128, 1024, 64, 32000
E = 2 * H  # encoder feature dim
BS = B // NCORES  # batch rows per core (16)
R = BS * L  # attention rows per core (1024)
HS = H // NCORES  # hidden slice per core (128)
VS = V // NCORES  # vocab slice per core (4000)
G4 = 4 * H

F32 = mybir.dt.float32
DT = mybir.dt.bfloat16
NP_DT = ml_dtypes.bfloat16

KE = E // 128  # 16 contraction chunks over encoder dim
KH = H // 128  # 8 contraction chunks over hidden dim
NEG_BIG = -1e30


def _bcast_dram(ap, parts):
    """AP that reads a [n] DRAM vector once per partition (partition step 0)."""
    return bass.AP(tensor=ap.tensor, offset=ap.offset, ap=[[0, parts]] + list(ap.ap))


def build_nc(with_gate_bias: bool, with_out_bias: bool):
    nc = bass.Bass(num_devices=NCORES)

    # ---- I/O ----
    encT_d = nc.dram_tensor("encT", [E, R], DT, kind="ExternalInput")
    encR_d = nc.dram_tensor("encR", [R, E], DT, kind="ExternalInput")
    uaT_d = nc.dram_tensor("uaT", [E, H], DT, kind="ExternalInput")
    waT_d = nc.dram_tensor("waT", [H, H], DT, kind="ExternalInput")
    h0T_d = nc.dram_tensor("h0T", [H, B], DT, kind="ExternalInput")
    h0Ts_d = nc.dram_tensor("h0Ts", [H, BS], DT, kind="ExternalInput")
    embT_d = nc.dram_tensor("embT", [H, B], DT, kind="ExternalInput")
    vlay_d = nc.dram_tensor("vlay", [128, KH], DT, kind="ExternalInput")
    ab_d = nc.dram_tensor("abbias", [128, KH], F32, kind="ExternalInput")
    maskb_d = nc.dram_tensor("maskb", [R], F32, kind="ExternalInput")
    wihT_d = nc.dram_tensor("wihT", [3 * H, 4 * HS], DT, kind="ExternalInput")
    whhT_d = nc.dram_tensor("whhT", [H, 4 * HS], DT, kind="ExternalInput")
    c0s_d = nc.dram_tensor("c0s", [B, HS], F32, kind="ExternalInput")
    woutT_d = nc.dram_tensor("woutT", [H, VS], DT, kind="ExternalInput")
    bg_d = nc.dram_tensor("bg", [4 * HS], F32, kind="ExternalInput")
    bo_d = nc.dram_tensor("bo", [VS], F32, kind="ExternalInput")

    attn_o = nc.dram_tensor("attn_o", [BS, L], F32, kind="ExternalOutput")
    h1_o = nc.dram_tensor("h1_o", [B, HS], F32, kind="ExternalOutput")
    c1_o = nc.dram_tensor("c1_o", [B, HS], F32, kind="ExternalOutput")
    logp_o = nc.dram_tensor("logp_o", [B, VS], F32, kind="ExternalOutput")

    # ---- internal DRAM for reshapes + collectives ----
    sc_d = nc.dram_tensor("sc_bounce", [BS, L], F32)
    cc_ctx_in = nc.dram_tensor("cc_ctx_in", [BS, E], DT)
    cc_ctx_out = nc.dram_tensor("cc_ctx_out", [B, E], DT, addr_space="Shared")
    cc_h1_in = nc.dram_tensor("cc_h1_in", [B, HS], DT)
    cc_h1_out = nc.dram_tensor("cc_h1_out", [NCORES, B, HS], DT, addr_space="Shared")
    cc_st_in = nc.dram_tensor("cc_st_in", [B, 2], F32)
    cc_st_out = nc.dram_tensor("cc_st_out", [NCORES, B, 2], F32, addr_space="Shared")

    groups = [list(range(NCORES))]

    from contextlib import ExitStack

    with tile.TileContext(nc) as tc, ExitStack() as es:
        const = es.enter_context(tc.tile_pool(name="const", bufs=1))
        work = es.enter_context(tc.tile_pool(name="work", bufs=3))

        # ---- persistent loads ----
        encT_sb = const.tile([128, KE, R], DT)
        nc.sync.dma_start(encT_sb[:], encT_d[:].rearrange("(k p) r -> p k r", p=128))
        uaT_sb = const.tile([128, KE, H], DT)
        nc.sync.dma_start(uaT_sb[:], uaT_d[:].rearrange("(k p) h -> p k h", p=128))
        waT_sb = const.tile([128, KH, H], DT)
        nc.sync.dma_start(waT_sb[:], waT_d[:].rearrange("(k p) h -> p k h", p=128))
        h0T_sb = const.tile([128, KH, B], DT)
        nc.sync.dma_start(h0T_sb[:], h0T_d[:].rearrange("(k p) b -> p k b", p=128))
        h0Ts_sb = const.tile([128, KH, BS], DT)
        nc.sync.dma_start(h0Ts_sb[:], h0Ts_d[:].rearrange("(k p) b -> p k b", p=128))
        embT_sb = const.tile([128, KH, B], DT)
        nc.sync.dma_start(embT_sb[:], embT_d[:].rearrange("(k p) b -> p k b", p=128))
        vlay_sb = const.tile([128, KH], DT)
        nc.sync.dma_start(vlay_sb[:], vlay_d[:])
        ab_sb = const.tile([128, KH], F32)
        nc.sync.dma_start(ab_sb[:], ab_d[:])
        maskb_sb = const.tile([1, R], F32)
        nc.sync.dma_start(maskb_sb[:], maskb_d[:][None, :])
        c0s_sb = const.tile([B, HS], F32)
        nc.sync.dma_start(c0s_sb[:], c0s_d[:])
        ident = const.tile([128, 128], DT)
        make_identity(nc, ident[:])

        if with_gate_bias:
            bg_sb = const.tile([128, 4 * HS], F32)
            nc.sync.dma_start(bg_sb[:], _bcast_dram(bg_d[:], 128))
        if with_out_bias:
            bo_sb = const.tile([128, VS], F32)
            nc.sync.dma_start(bo_sb[:], _bcast_dram(bo_d[:], 128))

        # ================= Phase 1: wa_sT = Wa @ h0_shard.T ================
        was_sb = const.tile([128, KH, BS], F32)
        with nc.named_scope("p1_wa"), tc.tile_pool(name="pwa", bufs=2, space="PSUM") as pwa:
            for m in range(KH):
                ps = pwa.tile([128, BS], F32)
                for k in range(KH):
                    nc.tensor.matmul(
                        ps[:],
                        waT_sb[:, k, m * 128 : (m + 1) * 128],
                        h0Ts_sb[:, k, :],
                        start=(k == 0),
                        stop=(k == KH - 1),
                    )
                nc.vector.tensor_copy(out=was_sb[:, m, :], in_=ps[:])

        # ====== Phase 2: ua_e + tanh + v-dot -> scores [1, R] ======
        NB = 8  # batch rows per 512-row chunk
        scores_sb = const.tile([1, R], F32)
        with (
            nc.named_scope("p2_ua"),
            tc.tile_pool(name="pua", bufs=2, space="PSUM") as pua,
            tc.tile_pool(name="psc", bufs=2, space="PSUM") as psc,
        ):
            for n in range(2):
                nsl = slice(n * 512, (n + 1) * 512)
                ps_sc = psc.tile([1, 512], F32)
                for m in range(KH):
                    ps_ua = pua.tile([128, 512], F32)
                    for k in range(KE):
                        nc.tensor.matmul(
                            ps_ua[:],
                            uaT_sb[:, k, m * 128 : (m + 1) * 128],
                            encT_sb[:, k, nsl],
                            start=(k == 0),
                            stop=(k == KE - 1),
                        )
                    pre = work.tile([128, 512], F32, tag="pre")
                    nc.vector.tensor_tensor(
                        out=pre[:].rearrange("p (b l) -> p b l", b=NB),
                        in0=ps_ua[:].rearrange("p (b l) -> p b l", b=NB),
                        in1=was_sb[:, m, n * NB : (n + 1) * NB][:, :, None].to_broadcast(
                            (128, NB, L)
                        ),
                        op=mybir.AluOpType.add,
                    )
                    th = work.tile([128, 512], DT, tag="tanh")
                    nc.scalar.activation(
                        out=th[:],
                        in_=pre[:],
                        func=mybir.ActivationFunctionType.Tanh,
                        bias=ab_sb[:, m : m + 1],
                        scale=1.0,
                    )
                    nc.tensor.matmul(
                        ps_sc[:],
                        vlay_sb[:, m : m + 1],
                        th[:],
                        start=(m == 0),
                        stop=(m == KH - 1),
                    )
                nc.vector.tensor_copy(out=scores_sb[:, nsl], in_=ps_sc[:])

        nc.vector.tensor_add(out=scores_sb[:], in0=scores_sb[:], in1=maskb_sb[:])

        sm_scope = nc.named_scope("p3_softmax")
        sm_scope.__enter__()
        # ====== Phase 3: softmax over L per batch row ======
        # reshape [1, R] -> [BS, L] through DRAM
        nc.sync.dma_start(out=sc_d[:].rearrange("b l -> (b l)")[None, :], in_=scores_sb[:])
        sc16 = work.tile([BS, L], F32, tag="sc16")
        nc.sync.dma_start(out=sc16[:], in_=sc_d[:])
        nmx16 = work.tile([BS, 1], F32, tag="nmx16")
        nc.vector.reduce_max(
            out=nmx16[:], in_=sc16[:], axis=mybir.AxisListType.X, negate=True
        )
        prob = work.tile([BS, L], F32, tag="prob")
        sumexp = work.tile([BS, 1], F32, tag="sumexp")
        nc.scalar.activation(
            out=prob[:],
            in_=sc16[:],
            func=mybir.ActivationFunctionType.Exp,
            bias=nmx16[:],
            scale=1.0,
            accum_out=sumexp[:],
        )
        rsum = work.tile([BS, 1], F32, tag="rsum")
        nc.vector.reciprocal(out=rsum[:], in_=sumexp[:])
        attn = work.tile([BS, L], F32, tag="attn")
        nc.vector.tensor_scalar_mul(out=attn[:], in0=prob[:], scalar1=rsum[:])
        nc.sync.dma_start(out=attn_o[:], in_=attn[:])
        attn_dt = work.tile([BS, L], DT, tag="attn_dt")
        nc.vector.tensor_copy(out=attn_dt[:], in_=attn[:])

        # attnT [L, BS] via PE transpose, then build the packed pair weights
        pairs = const.tile([128, BS], DT)
        nc.vector.memset(pairs[:], 0.0)
        with tc.tile_pool(name="ptr", bufs=2, space="PSUM") as ptr:
            ps_t = ptr.tile([L, BS], DT)
            nc.tensor.transpose(ps_t[:], attn_dt[:], ident[:BS, :BS])
            attnT = work.tile([L, BS], DT, tag="attnT")
            nc.vector.tensor_copy(out=attnT[:], in_=ps_t[:])
        nc.vector.tensor_copy(
            out=pairs[0:64, :].rearrange("p (j t) -> p j t", t=2)[:, :, 0],
            in_=attnT[:].rearrange("p (j t) -> p j t", t=2)[:, :, 0],
        )
        nc.vector.tensor_copy(
            out=pairs[64:128, :].rearrange("p (j t) -> p j t", t=2)[:, :, 1],
            in_=attnT[:].rearrange("p (j t) -> p j t", t=2)[:, :, 1],
        )

        sm_scope.__exit__(None, None, None)
        # ====== Phase 4: ctx = attn @ enc (pairs of batch rows packed in K) ======
        with (
            nc.named_scope("p4_ctx"),
            tc.tile_pool(name="encr", bufs=2) as encr,
            tc.tile_pool(name="ctxp", bufs=3) as ctxp,
            tc.tile_pool(name="pctx", bufs=2, space="PSUM") as pctx,
        ):
            for j in range(BS // 2):
                encRj = encr.tile([128, E], DT)
                nc.sync.dma_start(encRj[:], encR_d[j * 128 : (j + 1) * 128, :])
                ctxj = ctxp.tile([2, E], DT, tag="ctxj")
                for e in range(4):
                    esl = slice(e * 512, (e + 1) * 512)
                    ps_c = pctx.tile([2, 512], F32)
                    nc.tensor.matmul(
                        ps_c[:], pairs[:, 2 * j : 2 * j + 2], encRj[:, esl],
                        start=True, stop=True,
                    )
                    nc.vector.tensor_copy(out=ctxj[:, esl], in_=ps_c[:])
                nc.sync.dma_start(out=cc_ctx_in[2 * j : 2 * j + 2, :], in_=ctxj[:])
        with nc.named_scope("p4b_ag_ctx"):
            nc.gpsimd.collective_compute(
                "AllGather",
                mybir.AluOpType.bypass,
                replica_groups=groups,
                ins=[cc_ctx_in[:].opt()],
                outs=[cc_ctx_out[:].opt()],
            )
        # transposed loads: xct[:, k, :] = ctx_full[:, k*128:(k+1)*128].T
        xct_sb = const.tile([128, KE, B], DT)
        for k in range(KE):
            nc.sync.dma_start_transpose(
                xct_sb[:, k, :], cc_ctx_out[:, k * 128 : (k + 1) * 128]
            )

        # ====== Phase 5: LSTM gates (H-sliced) ======
        pre_g = work.tile([B, 4 * HS], F32, tag="pre_g")
        with (
            nc.named_scope("p5_gates"),
            tc.tile_pool(name="wg", bufs=3) as wg,
            tc.tile_pool(name="pg", bufs=1, space="PSUM") as pg,
        ):
            ps_g = pg.tile([B, 4 * HS], F32)
            nmm = 3 * KH + KE + KH
            i_mm = 0
            for k in range(KH):
                wt = wg.tile([128, 4 * HS], DT, tag="wt")
                nc.sync.dma_start(wt[:], wihT_d[k * 128 : (k + 1) * 128, :])
                nc.tensor.matmul(
                    ps_g[:], embT_sb[:, k, :], wt[:],
                    start=(i_mm == 0), stop=(i_mm == nmm - 1),
                )
                i_mm += 1
            for k in range(KE):
                wt = wg.tile([128, 4 * HS], DT, tag="wt")
                nc.sync.dma_start(wt[:], wihT_d[H + k * 128 : H + (k + 1) * 128, :])
                nc.tensor.matmul(
                    ps_g[:], xct_sb[:, k, :], wt[:],
                    start=(i_mm == 0), stop=(i_mm == nmm - 1),
                )
                i_mm += 1
            for k in range(KH):
                wt = wg.tile([128, 4 * HS], DT, tag="wt")
                nc.sync.dma_start(wt[:], whhT_d[k * 128 : (k + 1) * 128, :])
                nc.tensor.matmul(
                    ps_g[:], h0T_sb[:, k, :], wt[:],
                    start=(i_mm == 0), stop=(i_mm == nmm - 1),
                )
                i_mm += 1
            if with_gate_bias:
                nc.vector.tensor_add(out=pre_g[:], in0=ps_g[:], in1=bg_sb[:])
            else:
                nc.vector.tensor_copy(out=pre_g[:], in_=ps_g[:])

        ig = work.tile([B, HS], F32, tag="ig")
        fg = work.tile([B, HS], F32, tag="fg")
        gg = work.tile([B, HS], F32, tag="gg")
        og = work.tile([B, HS], F32, tag="og")
        Sig = mybir.ActivationFunctionType.Sigmoid
        Tanh = mybir.ActivationFunctionType.Tanh
        nc.scalar.activation(out=ig[:], in_=pre_g[:, 0 * HS : 1 * HS], func=Sig)
        nc.scalar.activation(out=fg[:], in_=pre_g[:, 1 * HS : 2 * HS], func=Sig)
        nc.scalar.activation(out=gg[:], in_=pre_g[:, 2 * HS : 3 * HS], func=Tanh)
        nc.scalar.activation(out=og[:], in_=pre_g[:, 3 * HS : 4 * HS], func=Sig)
        t1 = work.tile([B, HS], F32, tag="t1")
        nc.vector.tensor_mul(out=t1[:], in0=fg[:], in1=c0s_sb[:])
        t2 = work.tile([B, HS], F32, tag="t2")
        nc.vector.tensor_mul(out=t2[:], in0=ig[:], in1=gg[:])
        c1 = work.tile([B, HS], F32, tag="c1")
        nc.vector.tensor_add(out=c1[:], in0=t1[:], in1=t2[:])
        nc.sync.dma_start(out=c1_o[:], in_=c1[:])
        tc1 = work.tile([B, HS], F32, tag="tc1")
        nc.scalar.activation(out=tc1[:], in_=c1[:], func=Tanh)
        h1 = work.tile([B, HS], F32, tag="h1")
        nc.vector.tensor_mul(out=h1[:], in0=og[:], in1=tc1[:])
        nc.sync.dma_start(out=h1_o[:], in_=h1[:])
        h1_dt = work.tile([B, HS], DT, tag="h1_dt")
        nc.vector.tensor_copy(out=h1_dt[:], in_=h1[:])
        nc.sync.dma_start(out=cc_h1_in[:], in_=h1_dt[:])
        with nc.named_scope("p5b_ag_h1"):
            nc.gpsimd.collective_compute(
                "AllGather",
                mybir.AluOpType.bypass,
                replica_groups=groups,
                ins=[cc_h1_in[:].opt()],
                outs=[cc_h1_out[:].opt()],
            )
        h1T_sb = const.tile([128, KH, B], DT)
        for k in range(KH):
            nc.sync.dma_start_transpose(h1T_sb[:, k, :], cc_h1_out[k])

        # ====== Phase 6: logits + global log_softmax ======
        NL = VS // 8  # 500
        logits_sb = const.tile([B, VS], F32)
        mx8 = work.tile([B, 8], F32, tag="mx8")
        with (
            nc.named_scope("p6_logits"),
            tc.tile_pool(name="wo", bufs=2) as wo,
            tc.tile_pool(name="pl", bufs=1, space="PSUM") as pl,
        ):
            ps_l = [
                pl.tile([B, NL], F32, tag=f"pl{n}", name=f"pl{n}") for n in range(8)
            ]
            for k in range(KH):
                wt = wo.tile([128, VS], DT, tag="wo")
                nc.sync.dma_start(wt[:], woutT_d[k * 128 : (k + 1) * 128, :])
                for n in range(8):
                    nc.tensor.matmul(
                        ps_l[n][:],
                        h1T_sb[:, k, :],
                        wt[:, n * NL : (n + 1) * NL],
                        start=(k == 0),
                        stop=(k == KH - 1),
                    )
            for n in range(8):
                nsl = slice(n * NL, (n + 1) * NL)
                if with_out_bias:
                    nc.vector.tensor_tensor_reduce(
                        out=logits_sb[:, nsl],
                        in0=ps_l[n][:],
                        in1=bo_sb[:, nsl],
                        scale=1.0,
                        scalar=NEG_BIG,
                        op0=mybir.AluOpType.add,
                        op1=mybir.AluOpType.max,
                        accum_out=mx8[:, n : n + 1],
                    )
                else:
                    nc.vector.tensor_copy(out=logits_sb[:, nsl], in_=ps_l[n][:])
                    nc.vector.reduce_max(
                        out=mx8[:, n : n + 1],
                        in_=logits_sb[:, nsl],
                        axis=mybir.AxisListType.X,
                    )

        lsm_scope = nc.named_scope("p7_lsm")
        lsm_scope.__enter__()
        mxloc = work.tile([B, 1], F32, tag="mxloc")
        nc.vector.reduce_max(out=mxloc[:], in_=mx8[:], axis=mybir.AxisListType.X)
        nmx = work.tile([B, 1], F32, tag="nmx")
        nc.vector.tensor_scalar_mul(out=nmx[:], in0=mxloc[:], scalar1=-1.0)
        expbuf = const.tile([B, VS], F32)
        sloc = work.tile([B, 1], F32, tag="sloc")
        nc.scalar.activation(
            out=expbuf[:],
            in_=logits_sb[:],
            func=mybir.ActivationFunctionType.Exp,
            bias=nmx[:],
            scale=1.0,
            accum_out=sloc[:],
        )
        st = work.tile([B, 2], F32, tag="st")
        nc.vector.tensor_copy(out=st[:, 0:1], in_=mxloc[:])
        nc.vector.tensor_copy(out=st[:, 1:2], in_=sloc[:])
        nc.sync.dma_start(out=cc_st_in[:], in_=st[:])
        nc.gpsimd.collective_compute(
            "AllGather",
            mybir.AluOpType.bypass,
            replica_groups=groups,
            ins=[cc_st_in[:].opt()],
            outs=[cc_st_out[:].opt()],
        )
        st_all = work.tile([B, NCORES, 2], F32, tag="st_all")
        nc.sync.dma_start(out=st_all[:], in_=cc_st_out[:].rearrange("c p s -> p c s"))
        M = work.tile([B, 1], F32, tag="M")
        nc.vector.reduce_max(
            out=M[:], in_=st_all[:, :, 0], axis=mybir.AxisListType.X
        )
        d8 = work.tile([B, NCORES], F32, tag="d8")
        nc.vector.tensor_scalar(
            out=d8[:],
            in0=st_all[:, :, 0],
            scalar1=M[:],
            scalar2=None,
            op0=mybir.AluOpType.subtract,
        )
        e8 = work.tile([B, NCORES], F32, tag="e8")
        nc.scalar.activation(out=e8[:], in_=d8[:], func=mybir.ActivationFunctionType.Exp)
        t8 = work.tile([B, NCORES], F32, tag="t8")
        nc.vector.tensor_mul(out=t8[:], in0=e8[:], in1=st_all[:, :, 1])
        S = work.tile([B, 1], F32, tag="S")
        nc.vector.reduce_sum(out=S[:], in_=t8[:], axis=mybir.AxisListType.X)
        lnS = work.tile([B, 1], F32, tag="lnS")
        nc.scalar.activation(out=lnS[:], in_=S[:], func=mybir.ActivationFunctionType.Ln)
        lse = work.tile([B, 1], F32, tag="lse")
        nc.vector.tensor_add(out=lse[:], in0=lnS[:], in1=M[:])
        nc.vector.tensor_scalar(
            out=expbuf[:],
            in0=logits_sb[:],
            scalar1=lse[:],
            scalar2=None,
            op0=mybir.AluOpType.subtract,
        )
        nc.sync.dma_start(out=logp_o[:], in_=expbuf[:])
        lsm_scope.__exit__(None, None, None)

    return nc


_BUILD_CACHE = {}


def _get_nc(with_gate_bias: bool, with_out_bias: bool):
    key = (with_gate_bias, with_out_bias)
    if key not in _BUILD_CACHE:
        _BUILD_CACHE[key] = build_nc(*key)
    return _BUILD_CACHE[key]


def kernel(input_ids, h0, c0, encoder_outputs, mask,
           emb, Wa, ba, Ua, bUa, v, W_ih, W_hh, b_ih, b_hh, W_out, b_out):
    ids = np.asarray(input_ids).astype(np.int64)
    h0 = np.asarray(h0, dtype=np.float32)
    c0 = np.asarray(c0, dtype=np.float32)
    enc = np.asarray(encoder_outputs, dtype=np.float32)
    mask = np.asarray(mask)
    emb = np.asarray(emb, dtype=np.float32)
    Wa = np.asarray(Wa, dtype=np.float32)
    ba = np.asarray(ba, dtype=np.float32)
    Ua = np.asarray(Ua, dtype=np.float32)
    bUa = np.asarray(bUa, dtype=np.float32)
    v = np.asarray(v, dtype=np.float32)
    W_ih = np.asarray(W_ih, dtype=np.float32)
    W_hh = np.asarray(W_hh, dtype=np.float32)
    b_ih = np.asarray(b_ih, dtype=np.float32)
    b_hh = np.asarray(b_hh, dtype=np.float32)
    W_out = np.asarray(W_out, dtype=np.float32)
    b_out = np.asarray(b_out, dtype=np.float32)

    bg = b_ih + b_hh
    with_gate_bias = bool(np.any(bg != 0))
    with_out_bias = bool(np.any(b_out != 0))
    nc = _get_nc(with_gate_bias, with_out_bias)

    embedded = emb[ids]  # [B, H]
    embT = np.ascontiguousarray(embedded.T).astype(NP_DT)
    h0T = np.ascontiguousarray(h0.T).astype(NP_DT)
    waT = np.ascontiguousarray(Wa.T).astype(NP_DT)
    uaT = np.ascontiguousarray(Ua.T).astype(NP_DT)
    vlay = np.ascontiguousarray(v[0].reshape(KH, 128).T).astype(NP_DT)
    ab = np.ascontiguousarray((ba + bUa).reshape(KH, 128).T).astype(np.float32)
    wihT = np.ascontiguousarray(W_ih.T)  # [3H, 4H]
    whhT = np.ascontiguousarray(W_hh.T)  # [H, 4H]
    woutT = np.ascontiguousarray(W_out.T)  # [H, V]

    in_maps = []
    for c in range(NCORES):
        bsl = slice(c * BS, (c + 1) * BS)
        hsl_cols = np.concatenate(
            [np.arange(g * H + c * HS, g * H + (c + 1) * HS) for g in range(4)]
        )
        enc_c = enc[bsl].reshape(R, E)
        maskb = np.where(mask[bsl].reshape(R) == 0, np.float32(-1e9), np.float32(0.0))
        in_maps.append(
            {
                "encT": np.ascontiguousarray(enc_c.T).astype(NP_DT),
                "encR": enc_c.astype(NP_DT),
                "uaT": uaT,
                "waT": waT,
                "h0T": h0T,
                "h0Ts": np.ascontiguousarray(h0T[:, bsl]),
                "embT": embT,
                "vlay": vlay,
                "abbias": ab,
                "maskb": maskb,
                "wihT": np.ascontiguousarray(wihT[:, hsl_cols]).astype(NP_DT),
                "whhT": np.ascontiguousarray(whhT[:, hsl_cols]).astype(NP_DT),
                "c0s": np.ascontiguousarray(c0[:, c * HS : (c + 1) * HS]),
                "woutT": np.ascontiguousarray(woutT[:, c * VS : (c + 1) * VS]).astype(NP_DT),
                "bg": np.ascontiguousarray(bg[hsl_cols]),
                "bo": np.ascontiguousarray(b_out[c * VS : (c + 1) * VS]),
            }
        )

    trace = os.environ.get("KERNEL_TRACE", "0") == "1"
    res = run_bass_kernel_spmd(
        nc, in_maps, core_ids=list(range(NCORES)), trace=trace
    )
    if trace:
        kernel.last_result = res

    r = res.results
    log_probs = np.concatenate([r[c]["logp_o"] for c in range(NCORES)], axis=1)
    h1 = np.concatenate([r[c]["h1_o"] for c in range(NCORES)], axis=1)
    c1 = np.concatenate([r[c]["c1_o"] for c in range(NCORES)], axis=1)
    attn_w = np.concatenate([r[c]["attn_o"] for c in range(NCORES)], axis=0)
    return (log_probs, h1, c1, attn_w)


# revision 10
# speedup vs baseline: 1.4255x; 1.4255x over previous
"""Trainium2 Bass kernel for nn_AttnDecoderRNN (B=128, H=1024, L=64, V=32000).

Strategy across 8 NeuronCores:
  - Attention (the 34 GFLOP ua_e einsum + scores + softmax + context) is
    data-parallel over batch: each core owns 16 batch rows.
  - LSTM cell is tensor-parallel over the hidden dim: ctx is all-gathered,
    then each core computes a 128-wide H-slice of the gates/c1/h1 with the
    matching column slice of W_ih/W_hh.
  - Output projection is column-parallel over the vocab: h1 is all-gathered
    (transposed), each core computes logits for 4000 vocab columns, and a
    tiny stats all-gather turns local max/sumexp into the global log_softmax.

Matmul operands travel as bfloat16 (fp32 accumulation in PSUM); everything
else (softmax, LSTM elementwise, log-softmax) stays fp32.
"""
import os
import sys

if "/opt/trn_rl_repo" not in sys.path:
    sys.path.insert(0, "/opt/trn_rl_repo")

import ml_dtypes
import numpy as np
import orjson

# ---------------------------------------------------------------------------
# This container's walrus build only supports ONE semaphore wait per
# instruction ("Too many sync wait commands").  Split any instruction carrying
# k>1 waits into (k-1) standalone EventSemaphore waits on the same engine
# immediately before it (per-engine program order is preserved).
# ---------------------------------------------------------------------------


def _split_multiwaits(bir_bytes: bytes) -> bytes:
    j = orjson.loads(bir_bytes)
    counter = 0
    changed = False
    for func in j.get("functions", []):
        for bb in func.get("blocks", []):
            insts = bb.get("instructions", [])
            new_insts = []
            for ins in insts:
                si = ins.get("sync_info")
                if si:
                    waits = si.get("on_wait") or []
                    if len(waits) > 1:
                        changed = True
                        for w in waits[:-1]:
                            counter += 1
                            new_insts.append(
                                {
                                    "debug": ins.get("debug"),
                                    "engine": ins["engine"],
                                    "ins": [],
                                    "name": f"WSPLIT-{counter}",
                                    "opcode": "EventSemaphore",
                                    "outs": [],
                                    "sync_info": {"on_update": [], "on_wait": [w]},
                                }
                            )
                        si["on_wait"] = [waits[-1]]
                new_insts.append(ins)
            if len(new_insts) != len(insts):
                bb["instructions"] = new_insts
    return orjson.dumps(j) if changed else bir_bytes


def _install_birfix():
    import concourse.bass as bass

    if getattr(bass.Bass.to_json_bytes, "_wsplit_patched", False):
        return
    orig = bass.Bass.to_json_bytes

    def to_json_bytes(self):
        return _split_multiwaits(orig(self))

    to_json_bytes._wsplit_patched = True
    bass.Bass.to_json_bytes = to_json_bytes


_install_birfix()

import concourse.bass as bass
import concourse.tile as tile
from concourse import mybir
from concourse.bass_utils import run_bass_kernel_spmd
from concourse.masks import make_identity

# ---------------------------------------------------------------------------
# Problem constants (hardcoded per the harness contract).
# ---------------------------------------------------------------------------
NCORES = 8
B, H, L, V = 128, 1024, 64, 32000
E = 2 * H  # encoder feature dim
BS = B // NCORES  # batch rows per core (16)
R = BS * L  # attention rows per core (1024)
HS = H // NCORES  # hidden slice per core (128)
VS = V // NCORES  # vocab slice per core (4000)
G4 = 4 * H

F32 = mybir.dt.float32
DT = mybir.dt.bfloat16
NP_DT = ml_dtypes.bfloat16

KE = E // 128  # 16 contraction chunks over encoder dim
KH = H // 128  # 8 contraction chunks over hidden dim
NEG_BIG = -1e30


def _bcast_dram(ap, parts):
    """AP that reads a [n] DRAM vector once per partition (partition step 0)."""
    return bass.AP(tensor=ap.tensor, offset=ap.offset, ap=[[0, parts]] + list(ap.ap))


def build_nc(with_gate_bias: bool, with_out_bias: bool):
    nc = bass.Bass(num_devices=NCORES)

    # ---- I/O ----
    encT_d = nc.dram_tensor("encT", [E, R], DT, kind="ExternalInput")
    encR_d = nc.dram_tensor("encR", [R, E], DT, kind="ExternalInput")
    uaT_d = nc.dram_tensor("uaT", [E, H], DT, kind="ExternalInput")
    waT_d = nc.dram_tensor("waT", [H, H], DT, kind="ExternalInput")
    h0T_d = nc.dram_tensor("h0T", [H, B], DT, kind="ExternalInput")
    h0Ts_d = nc.dram_tensor("h0Ts", [H, BS], DT, kind="ExternalInput")
    embT_d = nc.dram_tensor("embT", [H, B], DT, kind="ExternalInput")
    vlay_d = nc.dram_tensor("vlay", [128, KH], DT, kind="ExternalInput")
    ab_d = nc.dram_tensor("abbias", [128, KH], F32, kind="ExternalInput")
    maskb_d = nc.dram_tensor("maskb", [R], F32, kind="ExternalInput")
    wihT_d = nc.dram_tensor("wihT", [3 * H, 4 * HS], DT, kind="ExternalInput")
    whhT_d = nc.dram_tensor("whhT", [H, 4 * HS], DT, kind="ExternalInput")
    c0s_d = nc.dram_tensor("c0s", [B, HS], F32, kind="ExternalInput")
    woutT_d = nc.dram_tensor("woutT", [H, VS], DT, kind="ExternalInput")
    bg_d = nc.dram_tensor("bg", [4 * HS], F32, kind="ExternalInput")
    bo_d = nc.dram_tensor("bo", [VS], F32, kind="ExternalInput")

    attn_o = nc.dram_tensor("attn_o", [BS, L], F32, kind="ExternalOutput")
    h1_o = nc.dram_tensor("h1_o", [B, HS], F32, kind="ExternalOutput")
    c1_o = nc.dram_tensor("c1_o", [B, HS], F32, kind="ExternalOutput")
    logp_o = nc.dram_tensor("logp_o", [B, VS], F32, kind="ExternalOutput")

    # ---- internal DRAM for reshapes + collectives ----
    sc_d = nc.dram_tensor("sc_bounce", [BS, L], F32)
    cc_ctx_in = nc.dram_tensor("cc_ctx_in", [BS, E], DT)
    cc_ctx_out = nc.dram_tensor("cc_ctx_out", [B, E], DT, addr_space="Shared")
    cc_h1_in = nc.dram_tensor("cc_h1_in", [B, HS], DT)
    cc_h1_out = nc.dram_tensor("cc_h1_out", [NCORES, B, HS], DT, addr_space="Shared")
    cc_st_in = nc.dram_tensor("cc_st_in", [B, 2], F32)
    cc_st_out = nc.dram_tensor("cc_st_out", [NCORES, B, 2], F32, addr_space="Shared")

    warm_in = nc.dram_tensor("warm_in", [BS, 16], F32)
    warm_out = nc.dram_tensor("warm_out", [B, 16], F32, addr_space="Shared")

    groups = [list(range(NCORES))]

    from contextlib import ExitStack

    with tile.TileContext(nc) as tc, ExitStack() as es:
        const = es.enter_context(tc.tile_pool(name="const", bufs=1))
        work = es.enter_context(tc.tile_pool(name="work", bufs=3))

        # Warmup collective: absorbs cross-core launch skew and TOPSP
        # first-collective overhead while the engines are busy with attention.
        with nc.named_scope("p0_warm_ag"):
            wtile = work.tile([BS, 16], F32, tag="wtile")
            nc.vector.memset(wtile[:], 0.0)
            nc.sync.dma_start(out=warm_in[:], in_=wtile[:])
            nc.gpsimd.collective_compute(
                "AllGather",
                mybir.AluOpType.bypass,
                replica_groups=groups,
                ins=[warm_in[:].opt()],
                outs=[warm_out[:].opt()],
            )

        # ---- persistent loads ----
        encT_sb = const.tile([128, KE, R], DT)
        nc.sync.dma_start(encT_sb[:], encT_d[:].rearrange("(k p) r -> p k r", p=128))
        uaT_sb = const.tile([128, KE, H], DT)
        nc.sync.dma_start(uaT_sb[:], uaT_d[:].rearrange("(k p) h -> p k h", p=128))
        waT_sb = const.tile([128, KH, H], DT)
        nc.sync.dma_start(waT_sb[:], waT_d[:].rearrange("(k p) h -> p k h", p=128))
        h0T_sb = const.tile([128, KH, B], DT)
        nc.sync.dma_start(h0T_sb[:], h0T_d[:].rearrange("(k p) b -> p k b", p=128))
        h0Ts_sb = const.tile([128, KH, BS], DT)
        nc.sync.dma_start(h0Ts_sb[:], h0Ts_d[:].rearrange("(k p) b -> p k b", p=128))
        embT_sb = const.tile([128, KH, B], DT)
        nc.sync.dma_start(embT_sb[:], embT_d[:].rearrange("(k p) b -> p k b", p=128))
        vlay_sb = const.tile([128, KH], DT)
        nc.sync.dma_start(vlay_sb[:], vlay_d[:])
        ab_sb = const.tile([128, KH], F32)
        nc.sync.dma_start(ab_sb[:], ab_d[:])
        maskb_sb = const.tile([1, R], F32)
        nc.sync.dma_start(maskb_sb[:], maskb_d[:][None, :])
        c0s_sb = const.tile([B, HS], F32)
        nc.sync.dma_start(c0s_sb[:], c0s_d[:])
        ident = const.tile([128, 128], DT)
        make_identity(nc, ident[:])

        if with_gate_bias:
            bg_sb = const.tile([128, 4 * HS], F32)
            nc.sync.dma_start(bg_sb[:], _bcast_dram(bg_d[:], 128))
        if with_out_bias:
            bo_sb = const.tile([128, VS], F32)
            nc.sync.dma_start(bo_sb[:], _bcast_dram(bo_d[:], 128))

        # ================= Phase 1: wa_sT = Wa @ h0_shard.T ================
        was_sb = const.tile([128, KH, BS], F32)
        with nc.named_scope("p1_wa"), tc.tile_pool(name="pwa", bufs=2, space="PSUM") as pwa:
            for m in range(KH):
                ps = pwa.tile([128, BS], F32)
                for k in range(KH):
                    nc.tensor.matmul(
                        ps[:],
                        waT_sb[:, k, m * 128 : (m + 1) * 128],
                        h0Ts_sb[:, k, :],
                        start=(k == 0),
                        stop=(k == KH - 1),
                    )
                nc.vector.tensor_copy(out=was_sb[:, m, :], in_=ps[:])

        # ====== Phase 2: ua_e + tanh + v-dot -> scores [1, R] ======
        NB = 8  # batch rows per 512-row chunk
        scores_sb = const.tile([1, R], F32)
        with (
            nc.named_scope("p2_ua"),
            tc.tile_pool(name="pua", bufs=2, space="PSUM") as pua,
            tc.tile_pool(name="psc", bufs=2, space="PSUM") as psc,
        ):
            for n in range(2):
                nsl = slice(n * 512, (n + 1) * 512)
                ps_sc = psc.tile([1, 512], F32)
                for m in range(KH):
                    ps_ua = pua.tile([128, 512], F32)
                    for k in range(KE):
                        nc.tensor.matmul(
                            ps_ua[:],
                            uaT_sb[:, k, m * 128 : (m + 1) * 128],
                            encT_sb[:, k, nsl],
                            start=(k == 0),
                            stop=(k == KE - 1),
                        )
                    pre = work.tile([128, 512], F32, tag="pre")
                    nc.vector.tensor_tensor(
                        out=pre[:].rearrange("p (b l) -> p b l", b=NB),
                        in0=ps_ua[:].rearrange("p (b l) -> p b l", b=NB),
                        in1=was_sb[:, m, n * NB : (n + 1) * NB][:, :, None].to_broadcast(
                            (128, NB, L)
                        ),
                        op=mybir.AluOpType.add,
                    )
                    th = work.tile([128, 512], DT, tag="tanh")
                    nc.scalar.activation(
                        out=th[:],
                        in_=pre[:],
                        func=mybir.ActivationFunctionType.Tanh,
                        bias=ab_sb[:, m : m + 1],
                        scale=1.0,
                    )
                    nc.tensor.matmul(
                        ps_sc[:],
                        vlay_sb[:, m : m + 1],
                        th[:],
                        start=(m == 0),
                        stop=(m == KH - 1),
                    )
                nc.vector.tensor_copy(out=scores_sb[:, nsl], in_=ps_sc[:])

        nc.vector.tensor_add(out=scores_sb[:], in0=scores_sb[:], in1=maskb_sb[:])

        sm_scope = nc.named_scope("p3_softmax")
        sm_scope.__enter__()
        # ====== Phase 3: softmax over L per batch row ======
        # reshape [1, R] -> [BS, L] through DRAM
        nc.sync.dma_start(out=sc_d[:].rearrange("b l -> (b l)")[None, :], in_=scores_sb[:])
        sc16 = work.tile([BS, L], F32, tag="sc16")
        nc.sync.dma_start(out=sc16[:], in_=sc_d[:])
        nmx16 = work.tile([BS, 1], F32, tag="nmx16")
        nc.vector.reduce_max(
            out=nmx16[:], in_=sc16[:], axis=mybir.AxisListType.X, negate=True
        )
        prob = work.tile([BS, L], F32, tag="prob")
        sumexp = work.tile([BS, 1], F32, tag="sumexp")
        nc.scalar.activation(
            out=prob[:],
            in_=sc16[:],
            func=mybir.ActivationFunctionType.Exp,
            bias=nmx16[:],
            scale=1.0,
            accum_out=sumexp[:],
        )
        rsum = work.tile([BS, 1], F32, tag="rsum")
        nc.vector.reciprocal(out=rsum[:], in_=sumexp[:])
        attn = work.tile([BS, L], F32, tag="attn")
        nc.vector.tensor_scalar_mul(out=attn[:], in0=prob[:], scalar1=rsum[:])
        nc.sync.dma_start(out=attn_o[:], in_=attn[:])
        attn_dt = work.tile([BS, L], DT, tag="attn_dt")
        nc.vector.tensor_copy(out=attn_dt[:], in_=attn[:])

        # attnT [L, BS] via PE transpose, then build the packed pair weights
        pairs = const.tile([128, BS], DT)
        nc.vector.memset(pairs[:], 0.0)
        with tc.tile_pool(name="ptr", bufs=2, space="PSUM") as ptr:
            ps_t = ptr.tile([L, BS], DT)
            nc.tensor.transpose(ps_t[:], attn_dt[:], ident[:BS, :BS])
            attnT = work.tile([L, BS], DT, tag="attnT")
            nc.vector.tensor_copy(out=attnT[:], in_=ps_t[:])
        nc.vector.tensor_copy(
            out=pairs[0:64, :].rearrange("p (j t) -> p j t", t=2)[:, :, 0],
            in_=attnT[:].rearrange("p (j t) -> p j t", t=2)[:, :, 0],
        )
        nc.vector.tensor_copy(
            out=pairs[64:128, :].rearrange("p (j t) -> p j t", t=2)[:, :, 1],
            in_=attnT[:].rearrange("p (j t) -> p j t", t=2)[:, :, 1],
        )

        sm_scope.__exit__(None, None, None)
        # ====== Phase 4: ctx = attn @ enc (pairs of batch rows packed in K) ======
        with (
            nc.named_scope("p4_ctx"),
            tc.tile_pool(name="encr", bufs=2) as encr,
            tc.tile_pool(name="ctxp", bufs=3) as ctxp,
            tc.tile_pool(name="pctx", bufs=2, space="PSUM") as pctx,
        ):
            for j in range(BS // 2):
                encRj = encr.tile([128, E], DT)
                nc.sync.dma_start(encRj[:], encR_d[j * 128 : (j + 1) * 128, :])
                ctxj = ctxp.tile([2, E], DT, tag="ctxj")
                for e in range(4):
                    esl = slice(e * 512, (e + 1) * 512)
                    ps_c = pctx.tile([2, 512], F32)
                    nc.tensor.matmul(
                        ps_c[:], pairs[:, 2 * j : 2 * j + 2], encRj[:, esl],
                        start=True, stop=True,
                    )
                    nc.vector.tensor_copy(out=ctxj[:, esl], in_=ps_c[:])
                nc.sync.dma_start(out=cc_ctx_in[2 * j : 2 * j + 2, :], in_=ctxj[:])
        with nc.named_scope("p4b_ag_ctx"):
            nc.gpsimd.collective_compute(
                "AllGather",
                mybir.AluOpType.bypass,
                replica_groups=groups,
                ins=[cc_ctx_in[:].opt()],
                outs=[cc_ctx_out[:].opt()],
            )
        # transposed loads: xct[:, k, :] = ctx_full[:, k*128:(k+1)*128].T
        xct_sb = const.tile([128, KE, B], DT)
        for k in range(KE):
            nc.sync.dma_start_transpose(
                xct_sb[:, k, :], cc_ctx_out[:, k * 128 : (k + 1) * 128]
            )

        # ====== Phase 5: LSTM gates (H-sliced) ======
        pre_g = work.tile([B, 4 * HS], F32, tag="pre_g")
        with (
            nc.named_scope("p5_gates"),
            tc.tile_pool(name="wg", bufs=3) as wg,
            tc.tile_pool(name="pg", bufs=1, space="PSUM") as pg,
        ):
            ps_g = pg.tile([B, 4 * HS], F32)
            nmm = 3 * KH + KE + KH
            i_mm = 0
            for k in range(KH):
                wt = wg.tile([128, 4 * HS], DT, tag="wt")
                nc.sync.dma_start(wt[:], wihT_d[k * 128 : (k + 1) * 128, :])
                nc.tensor.matmul(
                    ps_g[:], embT_sb[:, k, :], wt[:],
                    start=(i_mm == 0), stop=(i_mm == nmm - 1),
                )
                i_mm += 1
            for k in range(KE):
                wt = wg.tile([128, 4 * HS], DT, tag="wt")
                nc.sync.dma_start(wt[:], wihT_d[H + k * 128 : H + (k + 1) * 128, :])
                nc.tensor.matmul(
                    ps_g[:], xct_sb[:, k, :], wt[:],
                    start=(i_mm == 0), stop=(i_mm == nmm - 1),
                )
                i_mm += 1
            for k in range(KH):
                wt = wg.tile([128, 4 * HS], DT, tag="wt")
                nc.sync.dma_start(wt[:], whhT_d[k * 128 : (k + 1) * 128, :])
                nc.tensor.matmul(
                    ps_g[:], h0T_sb[:, k, :], wt[:],
                    start=(i_mm == 0), stop=(i_mm == nmm - 1),
                )
                i_mm += 1
            if with_gate_bias:
                nc.vector.tensor_add(out=pre_g[:], in0=ps_g[:], in1=bg_sb[:])
            else:
                nc.vector.tensor_copy(out=pre_g[:], in_=ps_g[:])

        ig = work.tile([B, HS], F32, tag="ig")
        fg = work.tile([B, HS], F32, tag="fg")
        gg = work.tile([B, HS], F32, tag="gg")
        og = work.tile([B, HS], F32, tag="og")
        Sig = mybir.ActivationFunctionType.Sigmoid
        Tanh = mybir.ActivationFunctionType.Tanh
        nc.scalar.activation(out=ig[:], in_=pre_g[:, 0 * HS : 1 * HS], func=Sig)
        nc.scalar.activation(out=fg[:], in_=pre_g[:, 1 * HS : 2 * HS], func=Sig)
        nc.scalar.activation(out=gg[:], in_=pre_g[:, 2 * HS : 3 * HS], func=Tanh)
        nc.scalar.activation(out=og[:], in_=pre_g[:, 3 * HS : 4 * HS], func=Sig)
        t1 = work.tile([B, HS], F32, tag="t1")
        nc.vector.tensor_mul(out=t1[:], in0=fg[:], in1=c0s_sb[:])
        t2 = work.tile([B, HS], F32, tag="t2")
        nc.vector.tensor_mul(out=t2[:], in0=ig[:], in1=gg[:])
        c1 = work.tile([B, HS], F32, tag="c1")
        nc.vector.tensor_add(out=c1[:], in0=t1[:], in1=t2[:])
        nc.sync.dma_start(out=c1_o[:], in_=c1[:])
        tc1 = work.tile([B, HS], F32, tag="tc1")
        nc.scalar.activation(out=tc1[:], in_=c1[:], func=Tanh)
        h1 = work.tile([B, HS], F32, tag="h1")
        nc.vector.tensor_mul(out=h1[:], in0=og[:], in1=tc1[:])
        nc.sync.dma_start(out=h1_o[:], in_=h1[:])
        h1_dt = work.tile([B, HS], DT, tag="h1_dt")
        nc.vector.tensor_copy(out=h1_dt[:], in_=h1[:])
        nc.sync.dma_start(out=cc_h1_in[:], in_=h1_dt[:])
        with nc.named_scope("p5b_ag_h1"):
            nc.gpsimd.collective_compute(
                "AllGather",
                mybir.AluOpType.bypass,
                replica_groups=groups,
                ins=[cc_h1_in[:].opt()],
                outs=[cc_h1_out[:].opt()],
            )
        h1T_sb = const.tile([128, KH, B], DT)
        for k in range(KH):
            nc.sync.dma_start_transpose(h1T_sb[:, k, :], cc_h1_out[k])

        # ====== Phase 6: logits + global log_softmax ======
        NL = VS // 8  # 500
        logits_sb = const.tile([B, VS], F32)
        mx8 = work.tile([B, 8], F32, tag="mx8")
        with (
            nc.named_scope("p6_logits"),
            tc.tile_pool(name="wo", bufs=2) as wo,
            tc.tile_pool(name="pl", bufs=1, space="PSUM") as pl,
        ):
            ps_l = [
                pl.tile([B, NL], F32, tag=f"pl{n}", name=f"pl{n}") for n in range(8)
            ]
            for k in range(KH):
                wt = wo.tile([128, VS], DT, tag="wo")
                nc.sync.dma_start(wt[:], woutT_d[k * 128 : (k + 1) * 128, :])
                for n in range(8):
                    nc.tensor.matmul(
                        ps_l[n][:],
                        h1T_sb[:, k, :],
                        wt[:, n * NL : (n + 1) * NL],
                        start=(k == 0),
                        stop=(k == KH - 1),
                    )
            for n in range(8):
                nsl = slice(n * NL, (n + 1) * NL)
                if with_out_bias:
                    nc.vector.tensor_tensor_reduce(
                        out=logits_sb[:, nsl],
                        in0=ps_l[n][:],
                        in1=bo_sb[:, nsl],
                        scale=1.0,
                        scalar=NEG_BIG,
                        op0=mybir.AluOpType.add,
                        op1=mybir.AluOpType.max,
                        accum_out=mx8[:, n : n + 1],
                    )
                else:
                    nc.vector.tensor_copy(out=logits_sb[:, nsl], in_=ps_l[n][:])
                    nc.vector.reduce_max(
                        out=mx8[:, n : n + 1],
                        in_=logits_sb[:, nsl],
                        axis=mybir.AxisListType.X,
                    )

        lsm_scope = nc.named_scope("p7_lsm")
        lsm_scope.__enter__()
        mxloc = work.tile([B, 1], F32, tag="mxloc")
        nc.vector.reduce_max(out=mxloc[:], in_=mx8[:], axis=mybir.AxisListType.X)
        nmx = work.tile([B, 1], F32, tag="nmx")
        nc.vector.tensor_scalar_mul(out=nmx[:], in0=mxloc[:], scalar1=-1.0)
        expbuf = const.tile([B, VS], F32)
        sloc = work.tile([B, 1], F32, tag="sloc")
        nc.scalar.activation(
            out=expbuf[:],
            in_=logits_sb[:],
            func=mybir.ActivationFunctionType.Exp,
            bias=nmx[:],
            scale=1.0,
            accum_out=sloc[:],
        )
        st = work.tile([B, 2], F32, tag="st")
        nc.vector.tensor_copy(out=st[:, 0:1], in_=mxloc[:])
        nc.vector.tensor_copy(out=st[:, 1:2], in_=sloc[:])
        nc.sync.dma_start(out=cc_st_in[:], in_=st[:])
        nc.gpsimd.collective_compute(
            "AllGather",
            mybir.AluOpType.bypass,
            replica_groups=groups,
            ins=[cc_st_in[:].opt()],
            outs=[cc_st_out[:].opt()],
        )
        st_all = work.tile([B, NCORES, 2], F32, tag="st_all")
        nc.sync.dma_start(out=st_all[:], in_=cc_st_out[:].rearrange("c p s -> p c s"))
        M = work.tile([B, 1], F32, tag="M")
        nc.vector.reduce_max(
            out=M[:], in_=st_all[:, :, 0], axis=mybir.AxisListType.X
        )
        d8 = work.tile([B, NCORES], F32, tag="d8")
        nc.vector.tensor_scalar(
            out=d8[:],
            in0=st_all[:, :, 0],
            scalar1=M[:],
            scalar2=None,
            op0=mybir.AluOpType.subtract,
        )
        e8 = work.tile([B, NCORES], F32, tag="e8")
        nc.scalar.activation(out=e8[:], in_=d8[:], func=mybir.ActivationFunctionType.Exp)
        t8 = work.tile([B, NCORES], F32, tag="t8")
        nc.vector.tensor_mul(out=t8[:], in0=e8[:], in1=st_all[:, :, 1])
        S = work.tile([B, 1], F32, tag="S")
        nc.vector.reduce_sum(out=S[:], in_=t8[:], axis=mybir.AxisListType.X)
        lnS = work.tile([B, 1], F32, tag="lnS")
        nc.scalar.activation(out=lnS[:], in_=S[:], func=mybir.ActivationFunctionType.Ln)
        lse = work.tile([B, 1], F32, tag="lse")
        nc.vector.tensor_add(out=lse[:], in0=lnS[:], in1=M[:])
        nc.vector.tensor_scalar(
            out=expbuf[:],
            in0=logits_sb[:],
            scalar1=lse[:],
            scalar2=None,
            op0=mybir.AluOpType.subtract,
        )
        nc.sync.dma_start(out=logp_o[:], in_=expbuf[:])
        lsm_scope.__exit__(None, None, None)

    return nc


_BUILD_CACHE = {}


def _get_nc(with_gate_bias: bool, with_out_bias: bool):
    key = (with_gate_bias, with_out_bias)
    if key not in _BUILD_CACHE:
        _BUILD_CACHE[key] = build_nc(*key)
    return _BUILD_CACHE[key]


def kernel(input_ids, h0, c0, encoder_outputs, mask,
           emb, Wa, ba, Ua, bUa, v, W_ih, W_hh, b_ih, b_hh, W_out, b_out):
    ids = np.asarray(input_ids).astype(np.int64)
    h0 = np.asarray(h0, dtype=np.float32)
    c0 = np.asarray(c0, dtype=np.float32)
    enc = np.asarray(encoder_outputs, dtype=np.float32)
    mask = np.asarray(mask)
    emb = np.asarray(emb, dtype=np.float32)
    Wa = np.asarray(Wa, dtype=np.float32)
    ba = np.asarray(ba, dtype=np.float32)
    Ua = np.asarray(Ua, dtype=np.float32)
    bUa = np.asarray(bUa, dtype=np.float32)
    v = np.asarray(v, dtype=np.float32)
    W_ih = np.asarray(W_ih, dtype=np.float32)
    W_hh = np.asarray(W_hh, dtype=np.float32)
    b_ih = np.asarray(b_ih, dtype=np.float32)
    b_hh = np.asarray(b_hh, dtype=np.float32)
    W_out = np.asarray(W_out, dtype=np.float32)
    b_out = np.asarray(b_out, dtype=np.float32)

    bg = b_ih + b_hh
    with_gate_bias = bool(np.any(bg != 0))
    with_out_bias = bool(np.any(b_out != 0))
    nc = _get_nc(with_gate_bias, with_out_bias)

    embedded = emb[ids]  # [B, H]
    embT = np.ascontiguousarray(embedded.T).astype(NP_DT)
    h0T = np.ascontiguousarray(h0.T).astype(NP_DT)
    waT = np.ascontiguousarray(Wa.T).astype(NP_DT)
    uaT = np.ascontiguousarray(Ua.T).astype(NP_DT)
    vlay = np.ascontiguousarray(v[0].reshape(KH, 128).T).astype(NP_DT)
    ab = np.ascontiguousarray((ba + bUa).reshape(KH, 128).T).astype(np.float32)
    wihT = np.ascontiguousarray(W_ih.T)  # [3H, 4H]
    whhT = np.ascontiguousarray(W_hh.T)  # [H, 4H]
    woutT = np.ascontiguousarray(W_out.T)  # [H, V]

    in_maps = []
    for c in range(NCORES):
        bsl = slice(c * BS, (c + 1) * BS)
        hsl_cols = np.concatenate(
            [np.arange(g * H + c * HS, g * H + (c + 1) * HS) for g in range(4)]
        )
        enc_c = enc[bsl].reshape(R, E)
        maskb = np.where(mask[bsl].reshape(R) == 0, np.float32(-1e9), np.float32(0.0))
        in_maps.append(
            {
                "encT": np.ascontiguousarray(enc_c.T).astype(NP_DT),
                "encR": enc_c.astype(NP_DT),
                "uaT": uaT,
                "waT": waT,
                "h0T": h0T,
                "h0Ts": np.ascontiguousarray(h0T[:, bsl]),
                "embT": embT,
                "vlay": vlay,
                "abbias": ab,
                "maskb": maskb,
                "wihT": np.ascontiguousarray(wihT[:, hsl_cols]).astype(NP_DT),
                "whhT": np.ascontiguousarray(whhT[:, hsl_cols]).astype(NP_DT),
                "c0s": np.ascontiguousarray(c0[:, c * HS : (c + 1) * HS]),
                "woutT": np.ascontiguousarray(woutT[:, c * VS : (c + 1) * VS]).astype(NP_DT),
                "bg": np.ascontiguousarray(bg[hsl_cols]),
                "bo": np.ascontiguousarray(b_out[c * VS : (c + 1) * VS]),
            }
        )

    trace = os.environ.get("KERNEL_TRACE", "0") == "1"
    res = run_bass_kernel_spmd(
        nc, in_maps, core_ids=list(range(NCORES)), trace=trace
    )
    if trace:
        kernel.last_result = res

    r = res.results
    log_probs = np.concatenate([r[c]["logp_o"] for c in range(NCORES)], axis=1)
    h1 = np.concatenate([r[c]["h1_o"] for c in range(NCORES)], axis=1)
    c1 = np.concatenate([r[c]["c1_o"] for c in range(NCORES)], axis=1)
    attn_w = np.concatenate([r[c]["attn_o"] for c in range(NCORES)], axis=0)
    return (log_probs, h1, c1, attn_w)


# revision 13
# speedup vs baseline: 1.5223x; 1.0679x over previous
"""Trainium2 Bass kernel for nn_AttnDecoderRNN (B=128, H=1024, L=64, V=32000).

Strategy across 8 NeuronCores:
  - Attention (the 34 GFLOP ua_e einsum + scores + softmax + context) is
    data-parallel over batch: each core owns 16 batch rows.
  - LSTM cell is tensor-parallel over the hidden dim: ctx is all-gathered,
    then each core computes a 128-wide H-slice of the gates/c1/h1 with the
    matching column slice of W_ih/W_hh.
  - Output projection is column-parallel over the vocab: h1 is all-gathered
    (pre-transposed), each core computes logits for 4000 vocab columns, and a
    tiny stats all-gather turns local max/sumexp into the global log_softmax.

Matmul operands travel as bfloat16 (fp32 accumulation in PSUM); everything
else (softmax, LSTM elementwise, log-softmax) stays fp32.
"""
import os
import sys

if "/opt/trn_rl_repo" not in sys.path:
    sys.path.insert(0, "/opt/trn_rl_repo")

import ml_dtypes
import numpy as np
import orjson

# ---------------------------------------------------------------------------
# This container's walrus build only supports ONE semaphore wait per
# instruction ("Too many sync wait commands").  Split any instruction carrying
# k>1 waits into (k-1) standalone EventSemaphore waits on the same engine
# immediately before it (per-engine program order is preserved).
# ---------------------------------------------------------------------------


def _split_multiwaits(bir_bytes: bytes) -> bytes:
    j = orjson.loads(bir_bytes)
    counter = 0
    changed = False
    for func in j.get("functions", []):
        for bb in func.get("blocks", []):
            insts = bb.get("instructions", [])
            new_insts = []
            for ins in insts:
                si = ins.get("sync_info")
                if si:
                    waits = si.get("on_wait") or []
                    if len(waits) > 1:
                        changed = True
                        for w in waits[:-1]:
                            counter += 1
                            new_insts.append(
                                {
                                    "debug": ins.get("debug"),
                                    "engine": ins["engine"],
                                    "ins": [],
                                    "name": f"WSPLIT-{counter}",
                                    "opcode": "EventSemaphore",
                                    "outs": [],
                                    "sync_info": {"on_update": [], "on_wait": [w]},
                                }
                            )
                        si["on_wait"] = [waits[-1]]
                new_insts.append(ins)
            if len(new_insts) != len(insts):
                bb["instructions"] = new_insts
    return orjson.dumps(j) if changed else bir_bytes


def _install_birfix():
    import concourse.bass as bass

    if getattr(bass.Bass.to_json_bytes, "_wsplit_patched", False):
        return
    orig = bass.Bass.to_json_bytes

    def to_json_bytes(self):
        return _split_multiwaits(orig(self))

    to_json_bytes._wsplit_patched = True
    bass.Bass.to_json_bytes = to_json_bytes


_install_birfix()

import concourse.bass as bass
import concourse.tile as tile
from concourse import mybir
from concourse.bass_utils import run_bass_kernel_spmd
from concourse.masks import make_identity

# ---------------------------------------------------------------------------
# Problem constants (hardcoded per the harness contract).
# ---------------------------------------------------------------------------
NCORES = 8
B, H, L, V = 128, 1024, 64, 32000
E = 2 * H  # encoder feature dim
BS = B // NCORES  # batch rows per core (16)
R = BS * L  # attention rows per core (1024)
HS = H // NCORES  # hidden slice per core (128)
VS = V // NCORES  # vocab slice per core (4000)

F32 = mybir.dt.float32
DT = mybir.dt.bfloat16
NP_DT = ml_dtypes.bfloat16

KE = E // 128  # 16 contraction chunks over encoder dim
KH = H // 128  # 8 contraction chunks over hidden dim
KX = 3 * H // 128  # 24 chunks over the LSTM x = [embedded; ctx] dim
NL = VS // 8  # logits free-dim chunk (500)


def _bcast_dram(ap, parts):
    """AP that reads a [n] DRAM vector once per partition (partition step 0)."""
    return bass.AP(tensor=ap.tensor, offset=ap.offset, ap=[[0, parts]] + list(ap.ap))


def build_nc(with_gate_bias: bool, with_out_bias: bool):
    nc = bass.Bass(num_devices=NCORES)

    # ---- I/O ----
    encT_d = nc.dram_tensor("encT", [E, R], DT, kind="ExternalInput")
    uaT_d = nc.dram_tensor("uaT", [E, H], DT, kind="ExternalInput")
    waT_d = nc.dram_tensor("waT", [H, H], DT, kind="ExternalInput")
    h0T_d = nc.dram_tensor("h0T", [H, B], DT, kind="ExternalInput")
    h0Ts_d = nc.dram_tensor("h0Ts", [H, BS], DT, kind="ExternalInput")
    embT_d = nc.dram_tensor("embT", [H, B], DT, kind="ExternalInput")
    vlay_d = nc.dram_tensor("vlay", [128, KH], DT, kind="ExternalInput")
    ab_d = nc.dram_tensor("abbias", [128, KH], F32, kind="ExternalInput")
    maskb_d = nc.dram_tensor("maskb", [R], F32, kind="ExternalInput")
    wihT_d = nc.dram_tensor("wihT", [3 * H, 4 * HS], DT, kind="ExternalInput")
    whhT_d = nc.dram_tensor("whhT", [H, 4 * HS], DT, kind="ExternalInput")
    c0s_d = nc.dram_tensor("c0s", [B, HS], F32, kind="ExternalInput")
    woutT_d = nc.dram_tensor("woutT", [H, VS], DT, kind="ExternalInput")
    bg_d = nc.dram_tensor("bg", [4 * HS], F32, kind="ExternalInput")
    bo_d = nc.dram_tensor("bo", [VS], F32, kind="ExternalInput")

    attn_o = nc.dram_tensor("attn_o", [BS, L], F32, kind="ExternalOutput")
    h1_o = nc.dram_tensor("h1_o", [B, HS], F32, kind="ExternalOutput")
    c1_o = nc.dram_tensor("c1_o", [B, HS], F32, kind="ExternalOutput")
    logp_o = nc.dram_tensor("logp_o", [B, VS], F32, kind="ExternalOutput")

    # ---- internal DRAM for reshapes + collectives ----
    sc_d = nc.dram_tensor("sc_bounce", [BS, L], F32)
    attn_d = nc.dram_tensor("attn_bounce", [BS, L], F32)
    cc_ctx_in = nc.dram_tensor("cc_ctx_in", [E, BS], DT)
    cc_ctx_out = nc.dram_tensor("cc_ctx_out", [NCORES, E, BS], DT, addr_space="Shared")
    cc_h1_in = nc.dram_tensor("cc_h1_in", [HS, B], DT)
    cc_h1_out = nc.dram_tensor("cc_h1_out", [NCORES, HS, B], DT, addr_space="Shared")
    cc_st_in = nc.dram_tensor("cc_st_in", [B, 2], F32)
    cc_st_out = nc.dram_tensor("cc_st_out", [NCORES, B, 2], F32, addr_space="Shared")
    warm_in = nc.dram_tensor("warm_in", [BS, 16], F32)
    warm_out = nc.dram_tensor("warm_out", [B, 16], F32, addr_space="Shared")

    groups = [list(range(NCORES))]
    # two independent HWDGE queue sets for DMA parallelism
    dmae = [nc.sync, nc.scalar]

    from contextlib import ExitStack

    with tile.TileContext(nc) as tc, ExitStack() as es:
        const = es.enter_context(tc.tile_pool(name="const", bufs=1))
        work = es.enter_context(tc.tile_pool(name="work", bufs=2))
        big1 = es.enter_context(tc.tile_pool(name="big1", bufs=1))

        # Warmup collective: absorbs cross-core launch skew and TOPSP
        # first-collective overhead while the engines are busy with attention.
        with nc.named_scope("p0_warm_ag"):
            wtile = work.tile([BS, 16], F32, tag="wtile")
            nc.vector.memset(wtile[:], 0.0)
            nc.gpsimd.dma_start(out=warm_in[:], in_=wtile[:])
            nc.gpsimd.collective_compute(
                "AllGather",
                mybir.AluOpType.bypass,
                replica_groups=groups,
                ins=[warm_in[:].opt()],
                outs=[warm_out[:].opt()],
            )

        # ---- persistent loads (small/urgent first, split across queues) ----
        h0Ts_sb = const.tile([128, KH, BS], DT)
        nc.sync.dma_start(h0Ts_sb[:], h0Ts_d[:].rearrange("(k p) b -> p k b", p=128))
        waT_sb = const.tile([128, KH, H], DT)
        for k in range(KH):
            dmae[k % 2].dma_start(
                waT_sb[:, k, :], waT_d[k * 128 : (k + 1) * 128, :]
            )
        vlay_sb = const.tile([128, KH], DT)
        nc.sync.dma_start(vlay_sb[:], vlay_d[:])
        ab_sb = const.tile([128, KH], F32)
        nc.scalar.dma_start(ab_sb[:], ab_d[:])
        maskb_sb = const.tile([1, R], F32)
        nc.sync.dma_start(maskb_sb[:], maskb_d[:][None, :])

        uaT_sb = const.tile([128, KE, H], DT)
        with tc.tile_pool(name="pencT", bufs=1) as pencT:
            encT_sb = pencT.tile([128, KE, R], DT)
            for k in range(KE):
                dmae[k % 2].dma_start(
                    encT_sb[:, k, :], encT_d[k * 128 : (k + 1) * 128, :]
                )
                dmae[(k + 1) % 2].dma_start(
                    uaT_sb[:, k, :], uaT_d[k * 128 : (k + 1) * 128, :]
                )

            h0T_sb = const.tile([128, KH, B], DT)
            nc.sync.dma_start(h0T_sb[:], h0T_d[:].rearrange("(k p) b -> p k b", p=128))
            embT_sb = const.tile([128, KH, B], DT)
            nc.scalar.dma_start(embT_sb[:], embT_d[:].rearrange("(k p) b -> p k b", p=128))
            wih_sb = const.tile([128, KX, 4 * HS], DT)
            for k in range(KX):
                dmae[k % 2].dma_start(
                    wih_sb[:, k, :], wihT_d[k * 128 : (k + 1) * 128, :]
                )
            whh_sb = const.tile([128, KH, 4 * HS], DT)
            for k in range(KH):
                dmae[k % 2].dma_start(
                    whh_sb[:, k, :], whhT_d[k * 128 : (k + 1) * 128, :]
                )
            c0s_sb = const.tile([B, HS], F32)
            nc.sync.dma_start(c0s_sb[:], c0s_d[:])
            ident = const.tile([128, 128], DT)
            make_identity(nc, ident[:])

            if with_gate_bias:
                bg_sb = const.tile([128, 4 * HS], F32)
                nc.sync.dma_start(bg_sb[:], _bcast_dram(bg_d[:], 128))
            if with_out_bias:
                bo_sb = const.tile([128, VS], F32)
                nc.scalar.dma_start(bo_sb[:], _bcast_dram(bo_d[:], 128))

            # ============ Phase 1: wa_sT = Wa @ h0_shard.T ============
            was_sb = const.tile([128, KH, BS], F32)
            with nc.named_scope("p1_wa"), tc.tile_pool(
                name="pwa", bufs=2, space="PSUM"
            ) as pwa:
                for m in range(KH):
                    ps = pwa.tile([128, BS], F32)
                    for k in range(KH):
                        nc.tensor.matmul(
                            ps[:],
                            waT_sb[:, k, m * 128 : (m + 1) * 128],
                            h0Ts_sb[:, k, :],
                            start=(k == 0),
                            stop=(k == KH - 1),
                        )
                    nc.vector.tensor_copy(out=was_sb[:, m, :], in_=ps[:])

            # ====== Phase 2: ua_e + tanh + v-dot -> scores [1, R] ======
            NB = 8  # batch rows per 512-row chunk
            scores_sb = big1.tile([1, R], F32)
            with (
                nc.named_scope("p2_ua"),
                tc.tile_pool(name="pua", bufs=2, space="PSUM") as pua,
                tc.tile_pool(name="psc", bufs=2, space="PSUM") as psc,
            ):
                for n in range(2):
                    nsl = slice(n * 512, (n + 1) * 512)
                    ps_sc = psc.tile([1, 512], F32)
                    for m in range(KH):
                        ps_ua = pua.tile([128, 512], F32)
                        for k in range(KE):
                            nc.tensor.matmul(
                                ps_ua[:],
                                uaT_sb[:, k, m * 128 : (m + 1) * 128],
                                encT_sb[:, k, nsl],
                                start=(k == 0),
                                stop=(k == KE - 1),
                            )
                        pre = work.tile([128, 512], F32, tag="pre")
                        nc.vector.tensor_tensor(
                            out=pre[:].rearrange("p (b l) -> p b l", b=NB),
                            in0=ps_ua[:].rearrange("p (b l) -> p b l", b=NB),
                            in1=was_sb[:, m, n * NB : (n + 1) * NB][
                                :, :, None
                            ].to_broadcast((128, NB, L)),
                            op=mybir.AluOpType.add,
                        )
                        th = work.tile([128, 512], DT, tag="tanh")
                        nc.scalar.activation(
                            out=th[:],
                            in_=pre[:],
                            func=mybir.ActivationFunctionType.Tanh,
                            bias=ab_sb[:, m : m + 1],
                            scale=1.0,
                        )
                        nc.tensor.matmul(
                            ps_sc[:],
                            vlay_sb[:, m : m + 1],
                            th[:],
                            start=(m == 0),
                            stop=(m == KH - 1),
                        )
                    # fold the -1e9 mask bias into the PSUM->SBUF copy
                    nc.vector.tensor_add(
                        out=scores_sb[:, nsl], in0=ps_sc[:], in1=maskb_sb[:, nsl]
                    )

            # ====== Phase 3: softmax over L per batch row ======
            sm = nc.named_scope("p3_softmax")
            sm.__enter__()
            nc.sync.dma_start(
                out=sc_d[:].rearrange("b l -> (b l)")[None, :], in_=scores_sb[:]
            )
            sc16 = work.tile([BS, L], F32, tag="sc16")
            nc.sync.dma_start(out=sc16[:], in_=sc_d[:])
            nmx16 = work.tile([BS, 1], F32, tag="nmx16")
            nc.vector.reduce_max(
                out=nmx16[:], in_=sc16[:], axis=mybir.AxisListType.X, negate=True
            )
            prob = work.tile([BS, L], F32, tag="prob")
            sumexp = work.tile([BS, 1], F32, tag="sumexp")
            nc.scalar.activation(
                out=prob[:],
                in_=sc16[:],
                func=mybir.ActivationFunctionType.Exp,
                bias=nmx16[:],
                scale=1.0,
                accum_out=sumexp[:],
            )
            rsum = work.tile([BS, 1], F32, tag="rsum")
            nc.vector.reciprocal(out=rsum[:], in_=sumexp[:])
            attn = work.tile([BS, L], F32, tag="attn")
            nc.vector.tensor_scalar_mul(out=attn[:], in0=prob[:], scalar1=rsum[:])
            nc.sync.dma_start(out=attn_o[:], in_=attn[:])
            nc.scalar.dma_start(out=attn_d[:], in_=attn[:])
            # broadcast attention weights to all 128 partitions (as [p, (b l)])
            attn_bcf = big1.tile([128, R], F32)
            nc.sync.dma_start(
                out=attn_bcf[:],
                in_=_bcast_dram(attn_d[:].rearrange("b l -> (b l)"), 128),
            )
            attn_bc = const.tile([128, R], DT)
            nc.vector.tensor_copy(out=attn_bc[:], in_=attn_bcf[:])
            sm.__exit__(None, None, None)

            # ====== Phase 4: ctxT[e, b] = sum_l enc[(b,l), e] * attn[b, l] ======
            # encT_sb already has e on partitions; contract l in the free dim
            # on the vector engine (bf16 4x mode), no PE needed.
            ctxT_f = const.tile([128, KE, BS], F32)
            with nc.named_scope("p4_ctx"):
                for k in range(KE):
                    tmp = work.tile([128, R], DT, tag="ctmp")
                    nc.vector.tensor_mul(
                        out=tmp[:], in0=encT_sb[:, k, :], in1=attn_bc[:]
                    )
                    nc.vector.tensor_reduce(
                        out=ctxT_f[:, k, :],
                        in_=tmp[:].rearrange("p (b l) -> p b l", b=BS),
                        axis=mybir.AxisListType.X,
                        op=mybir.AluOpType.add,
                    )
                ctxT_dt = const.tile([128, KE, BS], DT)
                nc.vector.tensor_copy(out=ctxT_dt[:], in_=ctxT_f[:])
                nc.sync.dma_start(
                    out=cc_ctx_in[:].rearrange("(k p) b -> p k b", p=128),
                    in_=ctxT_dt[:],
                )
            with nc.named_scope("p4b_ag_ctx"):
                nc.gpsimd.collective_compute(
                    "AllGather",
                    mybir.AluOpType.bypass,
                    replica_groups=groups,
                    ins=[cc_ctx_in[:].opt()],
                    outs=[cc_ctx_out[:].opt()],
                )
            # gather the transposed ctx of all cores: xct[:, k, :] = ctxT full
            xct_sb = const.tile([128, KE, B], DT)
            for k in range(KE):
                dmae[k % 2].dma_start(
                    xct_sb[:, k, :].rearrange("p (c b) -> p c b", c=NCORES),
                    cc_ctx_out[:, k * 128 : (k + 1) * 128, :].rearrange(
                        "c p b -> p c b"
                    ),
                )


        # ====== Phase 5: LSTM gates (H-sliced) ======
        pre_g = big1.tile([B, 4 * HS], F32)
        with (
            nc.named_scope("p5_gates"),
            tc.tile_pool(name="pg", bufs=1, space="PSUM") as pg,
        ):
            ps_g = pg.tile([B, 4 * HS], F32)
            nmm = KH + KH + KE
            i_mm = 0
            for k in range(KH):
                nc.tensor.matmul(
                    ps_g[:], embT_sb[:, k, :], wih_sb[:, k, :],
                    start=(i_mm == 0), stop=(i_mm == nmm - 1),
                )
                i_mm += 1
            for k in range(KH):
                nc.tensor.matmul(
                    ps_g[:], h0T_sb[:, k, :], whh_sb[:, k, :],
                    start=(i_mm == 0), stop=(i_mm == nmm - 1),
                )
                i_mm += 1
            for k in range(KE):
                nc.tensor.matmul(
                    ps_g[:], xct_sb[:, k, :], wih_sb[:, KH + k, :],
                    start=(i_mm == 0), stop=(i_mm == nmm - 1),
                )
                i_mm += 1
            if with_gate_bias:
                nc.vector.tensor_add(out=pre_g[:], in0=ps_g[:], in1=bg_sb[:])
            else:
                nc.vector.tensor_copy(out=pre_g[:], in_=ps_g[:])

        with nc.named_scope("p5c_lstm"):
            ig = work.tile([B, HS], F32, tag="ig")
            fg = work.tile([B, HS], F32, tag="fg")
            gg = work.tile([B, HS], F32, tag="gg")
            og = work.tile([B, HS], F32, tag="og")
            Sig = mybir.ActivationFunctionType.Sigmoid
            Tanh = mybir.ActivationFunctionType.Tanh
            nc.scalar.activation(out=ig[:], in_=pre_g[:, 0 * HS : 1 * HS], func=Sig)
            nc.scalar.activation(out=fg[:], in_=pre_g[:, 1 * HS : 2 * HS], func=Sig)
            nc.scalar.activation(out=gg[:], in_=pre_g[:, 2 * HS : 3 * HS], func=Tanh)
            nc.scalar.activation(out=og[:], in_=pre_g[:, 3 * HS : 4 * HS], func=Sig)
            t1 = work.tile([B, HS], F32, tag="t1")
            nc.vector.tensor_mul(out=t1[:], in0=fg[:], in1=c0s_sb[:])
            t2 = work.tile([B, HS], F32, tag="t2")
            nc.vector.tensor_mul(out=t2[:], in0=ig[:], in1=gg[:])
            c1 = work.tile([B, HS], F32, tag="c1")
            nc.vector.tensor_add(out=c1[:], in0=t1[:], in1=t2[:])
            nc.sync.dma_start(out=c1_o[:], in_=c1[:])
            tc1 = work.tile([B, HS], F32, tag="tc1")
            nc.scalar.activation(out=tc1[:], in_=c1[:], func=Tanh)
            h1 = work.tile([B, HS], F32, tag="h1")
            nc.vector.tensor_mul(out=h1[:], in0=og[:], in1=tc1[:])
            nc.sync.dma_start(out=h1_o[:], in_=h1[:])
            h1_dt = work.tile([B, HS], DT, tag="h1_dt")
            nc.vector.tensor_copy(out=h1_dt[:], in_=h1[:])
            # transpose before the all-gather so the gathered layout is h1T
            with tc.tile_pool(name="ph1", bufs=1, space="PSUM") as ph1:
                ps_t = ph1.tile([HS, B], DT)
                nc.tensor.transpose(ps_t[:], h1_dt[:], ident[:])
                h1Tc = work.tile([HS, B], DT, tag="h1Tc")
                nc.vector.tensor_copy(out=h1Tc[:], in_=ps_t[:])
            nc.sync.dma_start(out=cc_h1_in[:], in_=h1Tc[:])
        with nc.named_scope("p5b_ag_h1"):
            nc.gpsimd.collective_compute(
                "AllGather",
                mybir.AluOpType.bypass,
                replica_groups=groups,
                ins=[cc_h1_in[:].opt()],
                outs=[cc_h1_out[:].opt()],
            )
        h1T_sb = const.tile([128, KH, B], DT)
        nc.sync.dma_start(h1T_sb[:], cc_h1_out[:].rearrange("c p b -> p c b"))

        # ====== Phase 6: logits (vocab-sliced) + global log_softmax ======
        # W_out streams through the SBUF space freed by encT; k-outer keeps 8
        # open PSUM groups so each weight tile is used as soon as it lands.
        logits_bf = const.tile([B, VS], DT)
        mx8 = work.tile([B, 8], F32, tag="mx8")
        s8 = work.tile([B, 8], F32, tag="s8")

        def _logits_epilogue(n, ps):
            nsl = slice(n * NL, (n + 1) * NL)
            if with_out_bias:
                nc.vector.tensor_tensor_reduce(
                    out=logits_bf[:, nsl],
                    in0=ps[:],
                    in1=bo_sb[:, nsl],
                    scale=1.0,
                    scalar=-1e30,
                    op0=mybir.AluOpType.add,
                    op1=mybir.AluOpType.max,
                    accum_out=mx8[:, n : n + 1],
                )
            else:
                nc.vector.tensor_copy(out=logits_bf[:, nsl], in_=ps[:])
                nc.vector.reduce_max(
                    out=mx8[:, n : n + 1], in_=ps[:], axis=mybir.AxisListType.X
                )
            nmxn = work.tile([B, 1], F32, tag="nmxn", name="nmxn")
            nc.vector.tensor_scalar_mul(
                out=nmxn[:], in0=mx8[:, n : n + 1], scalar1=-1.0
            )
            expn = work.tile([B, NL], F32, tag="expn", name="expn")
            nc.scalar.activation(
                out=expn[:],
                in_=ps[:],
                func=mybir.ActivationFunctionType.Exp,
                bias=nmxn[:],
                scale=1.0,
                accum_out=s8[:, n : n + 1],
            )

        with (
            nc.named_scope("p6_logits"),
            tc.tile_pool(name="wo", bufs=3) as wo,
            tc.tile_pool(name="pl", bufs=1, space="PSUM") as pl,
        ):
            ps_l = [
                pl.tile([B, NL], F32, tag=f"pl{n}", name=f"pl{n}") for n in range(8)
            ]
            for k in range(KH):
                wt = wo.tile([128, VS], DT, tag="wo", name="wo")
                half = VS // 2
                dmae[0].dma_start(wt[:, :half], woutT_d[k * 128 : (k + 1) * 128, :half])
                dmae[1].dma_start(wt[:, half:], woutT_d[k * 128 : (k + 1) * 128, half:])
                for n in range(8):
                    nc.tensor.matmul(
                        ps_l[n][:],
                        h1T_sb[:, k, :],
                        wt[:, n * NL : (n + 1) * NL],
                        start=(k == 0),
                        stop=(k == KH - 1),
                    )
                    if k == KH - 1:
                        _logits_epilogue(n, ps_l[n])

        with nc.named_scope("p7_lsm"):
            # combine the 8 per-chunk (max, sumexp) into the core-local pair
            mxloc = work.tile([B, 1], F32, tag="mxloc")
            nc.vector.reduce_max(out=mxloc[:], in_=mx8[:], axis=mybir.AxisListType.X)
            d8 = work.tile([B, 8], F32, tag="d8")
            nc.vector.tensor_scalar(
                out=d8[:],
                in0=mx8[:],
                scalar1=mxloc[:],
                scalar2=None,
                op0=mybir.AluOpType.subtract,
            )
            e8 = work.tile([B, 8], F32, tag="e8")
            nc.scalar.activation(
                out=e8[:], in_=d8[:], func=mybir.ActivationFunctionType.Exp
            )
            t8 = work.tile([B, 8], F32, tag="t8")
            nc.vector.tensor_mul(out=t8[:], in0=e8[:], in1=s8[:])
            st = work.tile([B, 2], F32, tag="st")
            nc.vector.tensor_copy(out=st[:, 0:1], in_=mxloc[:])
            nc.vector.reduce_sum(
                out=st[:, 1:2], in_=t8[:], axis=mybir.AxisListType.X
            )
            nc.sync.dma_start(out=cc_st_in[:], in_=st[:])
            nc.gpsimd.collective_compute(
                "AllGather",
                mybir.AluOpType.bypass,
                replica_groups=groups,
                ins=[cc_st_in[:].opt()],
                outs=[cc_st_out[:].opt()],
            )
            st_all = work.tile([B, NCORES, 2], F32, tag="st_all")
            nc.sync.dma_start(
                out=st_all[:], in_=cc_st_out[:].rearrange("c p s -> p c s")
            )
            M = work.tile([B, 1], F32, tag="M")
            nc.vector.reduce_max(
                out=M[:], in_=st_all[:, :, 0], axis=mybir.AxisListType.X
            )
            dg = work.tile([B, NCORES], F32, tag="dg")
            nc.vector.tensor_scalar(
                out=dg[:],
                in0=st_all[:, :, 0],
                scalar1=M[:],
                scalar2=None,
                op0=mybir.AluOpType.subtract,
            )
            eg = work.tile([B, NCORES], F32, tag="eg")
            nc.scalar.activation(
                out=eg[:], in_=dg[:], func=mybir.ActivationFunctionType.Exp
            )
            tg = work.tile([B, NCORES], F32, tag="tg")
            nc.vector.tensor_mul(out=tg[:], in0=eg[:], in1=st_all[:, :, 1])
            S = work.tile([B, 1], F32, tag="S")
            nc.vector.reduce_sum(out=S[:], in_=tg[:], axis=mybir.AxisListType.X)
            lnS = work.tile([B, 1], F32, tag="lnS")
            nc.scalar.activation(
                out=lnS[:], in_=S[:], func=mybir.ActivationFunctionType.Ln
            )
            lse = work.tile([B, 1], F32, tag="lse")
            nc.vector.tensor_add(out=lse[:], in0=lnS[:], in1=M[:])
            for n in range(8):
                nsl = slice(n * NL, (n + 1) * NL)
                outn = work.tile([B, NL], F32, tag="outn")
                nc.vector.tensor_scalar(
                    out=outn[:],
                    in0=logits_bf[:, nsl],
                    scalar1=lse[:],
                    scalar2=None,
                    op0=mybir.AluOpType.subtract,
                )
                dmae[n % 2].dma_start(out=logp_o[:, nsl], in_=outn[:])

    return nc


_BUILD_CACHE = {}


def _get_nc(with_gate_bias: bool, with_out_bias: bool):
    key = (with_gate_bias, with_out_bias)
    if key not in _BUILD_CACHE:
        _BUILD_CACHE[key] = build_nc(*key)
    return _BUILD_CACHE[key]


def kernel(input_ids, h0, c0, encoder_outputs, mask,
           emb, Wa, ba, Ua, bUa, v, W_ih, W_hh, b_ih, b_hh, W_out, b_out):
    ids = np.asarray(input_ids).astype(np.int64)
    h0 = np.asarray(h0, dtype=np.float32)
    c0 = np.asarray(c0, dtype=np.float32)
    enc = np.asarray(encoder_outputs, dtype=np.float32)
    mask = np.asarray(mask)
    emb = np.asarray(emb, dtype=np.float32)
    Wa = np.asarray(Wa, dtype=np.float32)
    ba = np.asarray(ba, dtype=np.float32)
    Ua = np.asarray(Ua, dtype=np.float32)
    bUa = np.asarray(bUa, dtype=np.float32)
    v = np.asarray(v, dtype=np.float32)
    W_ih = np.asarray(W_ih, dtype=np.float32)
    W_hh = np.asarray(W_hh, dtype=np.float32)
    b_ih = np.asarray(b_ih, dtype=np.float32)
    b_hh = np.asarray(b_hh, dtype=np.float32)
    W_out = np.asarray(W_out, dtype=np.float32)
    b_out = np.asarray(b_out, dtype=np.float32)

    bg = b_ih + b_hh
    with_gate_bias = bool(np.any(bg != 0))
    with_out_bias = bool(np.any(b_out != 0))
    nc = _get_nc(with_gate_bias, with_out_bias)

    embedded = emb[ids]  # [B, H]
    embT = np.ascontiguousarray(embedded.T).astype(NP_DT)
    h0T = np.ascontiguousarray(h0.T).astype(NP_DT)
    waT = np.ascontiguousarray(Wa.T).astype(NP_DT)
    uaT = np.ascontiguousarray(Ua.T).astype(NP_DT)
    vlay = np.ascontiguousarray(v[0].reshape(KH, 128).T).astype(NP_DT)
    ab = np.ascontiguousarray((ba + bUa).reshape(KH, 128).T).astype(np.float32)
    wihT = np.ascontiguousarray(W_ih.T)  # [3H, 4H]
    whhT = np.ascontiguousarray(W_hh.T)  # [H, 4H]
    woutT = np.ascontiguousarray(W_out.T)  # [H, V]

    in_maps = []
    for c in range(NCORES):
        bsl = slice(c * BS, (c + 1) * BS)
        hsl_cols = np.concatenate(
            [np.arange(g * H + c * HS, g * H + (c + 1) * HS) for g in range(4)]
        )
        enc_c = enc[bsl].reshape(R, E)
        maskb = np.where(mask[bsl].reshape(R) == 0, np.float32(-1e9), np.float32(0.0))
        in_maps.append(
            {
                "encT": np.ascontiguousarray(enc_c.T).astype(NP_DT),
                "uaT": uaT,
                "waT": waT,
                "h0T": h0T,
                "h0Ts": np.ascontiguousarray(h0T[:, bsl]),
                "embT": embT,
                "vlay": vlay,
                "abbias": ab,
                "maskb": maskb,
                "wihT": np.ascontiguousarray(wihT[:, hsl_cols]).astype(NP_DT),
                "whhT": np.ascontiguousarray(whhT[:, hsl_cols]).astype(NP_DT),
                "c0s": np.ascontiguousarray(c0[:, c * HS : (c + 1) * HS]),
                "woutT": np.ascontiguousarray(woutT[:, c * VS : (c + 1) * VS]).astype(NP_DT),
                "bg": np.ascontiguousarray(bg[hsl_cols]),
                "bo": np.ascontiguousarray(b_out[c * VS : (c + 1) * VS]),
            }
        )

    trace = os.environ.get("KERNEL_TRACE", "0") == "1"
    res = run_bass_kernel_spmd(
        nc, in_maps, core_ids=list(range(NCORES)), trace=trace
    )
    if trace:
        kernel.last_result = res

    r = res.results
    log_probs = np.concatenate([r[c]["logp_o"] for c in range(NCORES)], axis=1)
    h1 = np.concatenate([r[c]["h1_o"] for c in range(NCORES)], axis=1)
    c1 = np.concatenate([r[c]["c1_o"] for c in range(NCORES)], axis=1)
    attn_w = np.concatenate([r[c]["attn_o"] for c in range(NCORES)], axis=0)
    return (log_probs, h1, c1, attn_w)
